# revision 14
# baseline (speedup 1.0000x reference)
"""Trainium2 Bass kernel for nn_DA_conv (dense_cnn).

Model (per batch element b, channels c):
  kern = leaky(d @ kW1.T) @ kW2.T            -> per-(b,c) 3x3 depthwise filter
  dw   = depthwise_conv3x3(x, kern), pad=1   (cross-correlation)
  act  = leaky(dw)
  out  = conv1x1(act, convW) + convB + x * sigmoid-attention(d)

Sharding: data-parallel over batch B=16 across 8 cores (2 images/core).
Per-core layout: 128 SBUF partitions = (2 images x 64 channels); spatial
plane stored flat with 1-pixel zero padding: rows of 130 floats.

Engine split per 11-row band:
  - PE  : 9 depthwise taps as diag-matrix matmuls (bf16) accumulated in
          PSUM + 1x1 conv as block-diag matmul + bias via K=1 ones-matmul
  - ACT : fp32->bf16 cast of the band, leaky (Prelu alpha=0.1) PSUM->SBUF
  - DVE : final combine (x*att + psum) evacuation; plus a few whole bands
          computed on DVE via scalar_tensor_tensor taps to offload PE
The tiny d-MLPs (kern, att) run on-device with fp32 matmuls; the
per-(b,c)-filter rearranges go through small DRAM scratch roundtrips.
"""
import numpy as np
import ml_dtypes

import concourse.bacc as bacc
import concourse.bass as bass
import concourse.mybir as mybir
import concourse.tile as tile
from concourse.bass_utils import run_bass_kernel_spmd
from concourse.masks import make_identity

F32 = mybir.dt.float32
BF16 = mybir.dt.bfloat16
AF = mybir.ActivationFunctionType
ALU = mybir.AluOpType

B, C, H, W = 16, 64, 128, 128
NCORES = 8
BL = B // NCORES          # images per core (2)
P = BL * C                # partitions used (128)
WP = W + 2                # padded row length (130)
NEG = 0.1                 # leaky slope

BAND = 11                 # interior rows per band
DVE_BANDS = (4, 11)       # bands whose depthwise runs on DVE instead of PE

_CACHE = {}


def _bands():
    out = []
    p0 = 0
    while p0 < H:
        nr = min(BAND, H - p0)
        out.append((p0, nr))
        p0 += nr
    return out


def _tiles(group=3):
    """Group sub-bands into DMA super-tiles of `group` bands each."""
    bands = _bands()
    out = []
    for i in range(0, len(bands), group):
        chunk = bands[i:i + group]
        row0 = chunk[0][0]
        nrows = sum(nr for _, nr in chunk)
        out.append((row0, nrows))
    return out


def _chunks(span):
    return [(cs, min(512, span - cs)) for cs in range(0, span, 512)]


def _build():
    nc = bacc.Bacc("TRN2", target_bir_lowering=False, debug=False)

    x_d = nc.dram_tensor("x", [BL, C, H, W], BF16, kind="ExternalInput")
    # packed [64, 650] = kW1T | kW2T | caW1T | dT  (all fp32, 64 rows)
    wpk_d = nc.dram_tensor("wpack", [C, 650], F32, kind="ExternalInput")
    caW2T_d = nc.dram_tensor("caW2T", [8, C], F32, kind="ExternalInput")
    cwbd_d = nc.dram_tensor("convWbd16", [P, P], BF16, kind="ExternalInput")
    cbf_d = nc.dram_tensor("convB2f", [P, 1], F32, kind="ExternalInput")
    out_d = nc.dram_tensor("out", [BL, C, H, W], F32, kind="ExternalOutput")

    with tile.TileContext(nc) as tc:
        with (
            tc.tile_pool(name="consts", bufs=1) as consts,
            tc.tile_pool(name="xb", bufs=3) as xbp,
            tc.tile_pool(name="actb", bufs=3) as actbp,
            tc.tile_pool(name="accb", bufs=2) as accbp,
            tc.tile_pool(name="outb", bufs=3) as outbp,
            tc.tile_pool(name="psA", bufs=2, space="PSUM") as psA,
            tc.tile_pool(name="psB", bufs=2, space="PSUM") as psB,
            tc.tile_pool(name="dram", bufs=1, space="DRAM") as dram,
        ):
            # ---- load weights/inputs that persist ----
            wpk = consts.tile([C, 650], F32)
            caW2T = consts.tile([8, C], F32)
            cwbd = consts.tile([P, P], BF16)
            cbf = consts.tile([P, 1], F32)
            nc.sync.dma_start(out=wpk, in_=wpk_d.ap())
            nc.sync.dma_start(out=caW2T, in_=caW2T_d.ap())
            nc.sync.dma_start(out=cwbd, in_=cwbd_d.ap())
            nc.sync.dma_start(out=cbf, in_=cbf_d.ap())
            kW1T = wpk[:, 0:64]
            kW2T = wpk[:, 64:640]
            caW1T = wpk[:, 640:648]
            dT = wpk[:, 648:650]

            ident = consts.tile([P, P], F32)
            make_identity(nc, ident)

            # ---- kern MLP: kern = leaky(d @ kW1.T) @ kW2.T ----
            h1p = psB.tile([C, BL], F32, tag="B")
            nc.tensor.matmul(h1p, kW1T, dT, start=True, stop=True)
            h1 = consts.tile([C, BL], F32, tag="h1")
            nc.scalar.activation(h1, h1p, AF.Prelu, alpha=NEG)

            kernp = psB.tile([P, 10], F32, tag="B")
            for j in range(5):
                m = min(128, C * 9 - 128 * j)
                nc.tensor.matmul(kernp[0:m, 2 * j:2 * j + 2],
                                 kW2T[:, 128 * j:128 * j + m], h1,
                                 start=True, stop=True)
            kernf = consts.tile([P, 10], F32, tag="kernf")
            nc.scalar.copy(kernf, kernp)

            # roundtrip through DRAM to re-lay kern as [(b,c), tap]
            skern = dram.tile([C * 9 * BL], F32)
            sk_t = skern.tensor
            # flat scratch address = j*2 + b with j = jc*128 + p (j < 576)
            nc.sync.dma_start(
                out=bass.AP(tensor=sk_t, offset=skern.offset,
                            ap=[[2, 128], [256, 4], [1, 2]]),
                in_=kernf[:, 0:8].rearrange("p (j b) -> p j b", b=2))
            nc.sync.dma_start(
                out=bass.AP(tensor=sk_t, offset=skern.offset + 1024,
                            ap=[[2, 64], [1, 2]]),
                in_=kernf[0:64, 8:10])
            kern_pp = consts.tile([P, 9], F32, tag="kern_pp")
            for b in range(2):
                nc.sync.dma_start(
                    out=kern_pp[64 * b:64 * (b + 1), :],
                    in_=bass.AP(tensor=sk_t, offset=skern.offset + b,
                                ap=[[18, 64], [2, 9]]))

            # diag tap matrices (bf16): diag16[:, t*128:(t+1)*128] = I * kern_t
            diag16 = consts.tile([P, 9 * P], BF16, tag="diag16")
            for t in range(9):
                nc.vector.tensor_scalar(diag16[:, P * t:P * (t + 1)], ident,
                                        kern_pp[:, t:t + 1], None, ALU.mult)

            # ---- attention MLP: att = sigmoid(leaky(d @ caW1.T) @ caW2.T) ----
            a1p = psB.tile([8, BL], F32, tag="B")
            nc.tensor.matmul(a1p, caW1T, dT, start=True, stop=True)
            a1 = consts.tile([8, BL], F32, tag="a1")
            nc.scalar.activation(a1, a1p, AF.Prelu, alpha=NEG)
            attp = psB.tile([C, BL], F32, tag="B")
            nc.tensor.matmul(attp, caW2T, a1, start=True, stop=True)
            atts = consts.tile([C, BL], F32, tag="atts")
            nc.scalar.activation(atts, attp, AF.Sigmoid)

            satt = dram.tile([P], F32)
            nc.sync.dma_start(
                out=bass.AP(tensor=satt.tensor, offset=satt.offset,
                            ap=[[1, 64], [64, 2]]),
                in_=atts)
            att_pp = consts.tile([P, 1], F32, tag="att_pp")
            nc.sync.dma_start(
                out=att_pp,
                in_=bass.AP(tensor=satt.tensor, offset=satt.offset,
                            ap=[[1, 128], [0, 1]]))
            attd16 = consts.tile([P, P], BF16, tag="attd16")
            nc.vector.tensor_scalar(attd16, ident, att_pp[:, 0:1], None,
                                    ALU.mult)

            # ---- main loop: DMA super-tiles of TROWS rows, compute
            # ---- sub-bands of BAND rows inside each tile ----
            bands = _bands()
            bi = 0
            for (row0, tnr) in _tiles():
                R = tnr + 2                   # padded rows in this DMA tile
                xb = xbp.tile([P, R * WP], BF16, tag="xb")
                xbv = xb.rearrange("p (r w) -> p r w", w=WP)
                # zero the left/right padding columns
                nc.gpsimd.memset(xbv[:, :, 0:1], 0.0)
                nc.gpsimd.memset(xbv[:, :, W + 1:W + 2], 0.0)
                # zero top/bottom padding rows (first/last tile only)
                r_lo = max(0, 1 - row0)
                r_hi = min(R, 129 - row0)
                if r_lo > 0:
                    nc.gpsimd.memset(xbv[:, 0:r_lo, 1:W + 1], 0.0)
                if r_hi < R:
                    nc.gpsimd.memset(xbv[:, r_hi:R, 1:W + 1], 0.0)
                nc.sync.dma_start(
                    out=xbv[:, r_lo:r_hi, 1:W + 1],
                    in_=x_d.ap().rearrange("b c h w -> (b c) h w")
                    [:, row0 + r_lo - 1:row0 + r_hi - 1, :])

                outb = outbp.tile([P, tnr * WP], F32, tag="outb")

                while bi < len(bands) and bands[bi][0] < row0 + tnr:
                    p0, nr = bands[bi]
                    span = (nr - 1) * WP + W
                    # offset of this sub-band's first interior output in xb
                    base = (p0 - row0 + 1) * WP + 1
                    obase = (p0 - row0) * WP   # ... and in outb

                    actb = actbp.tile([P, span], BF16, tag="actb")
                    if bi not in DVE_BANDS:
                        # PE depthwise: 9 diag matmuls per window
                        pa = psA.tile([P, span], F32, tag="A")
                        for (cs, wn) in _chunks(span):
                            for t in range(9):
                                ky, kx = t // 3, t % 3
                                off = base + (ky - 1) * WP + (kx - 1) + cs
                                nc.tensor.matmul(
                                    pa[:, cs:cs + wn],
                                    diag16[:, P * t:P * (t + 1)],
                                    xb[:, off:off + wn],
                                    start=(t == 0), stop=(t == 8))
                        nc.scalar.activation(actb, pa, AF.Prelu, alpha=NEG)
                    else:
                        # DVE depthwise: scalar_tensor_tensor tap chain
                        acc = accbp.tile([P, span], F32, tag="acc")
                        for t in range(9):
                            ky, kx = t // 3, t % 3
                            off = base + (ky - 1) * WP + (kx - 1)
                            src = xb[:, off:off + span]
                            if t == 0:
                                nc.vector.tensor_scalar(
                                    acc, src, kern_pp[:, 0:1], None, ALU.mult)
                            else:
                                nc.vector.scalar_tensor_tensor(
                                    acc, src, kern_pp[:, t:t + 1], acc,
                                    op0=ALU.mult, op1=ALU.add)
                        nc.vector.scalar_tensor_tensor(
                            actb, acc, NEG, acc, op0=ALU.mult, op1=ALU.max)

                    # 1x1 conv + att*x residual into PSUM, evac on ACT
                    # with convB as per-partition bias
                    for (cs, wn) in _chunks(span):
                        pb = psB.tile([P, 512], F32, tag="B")
                        nc.tensor.matmul(pb[:, 0:wn], cwbd,
                                         actb[:, cs:cs + wn],
                                         start=True, stop=False)
                        nc.tensor.matmul(pb[:, 0:wn], attd16,
                                         xb[:, base + cs:base + cs + wn],
                                         start=False, stop=True)
                        nc.scalar.activation(
                            outb[:, obase + cs:obase + cs + wn], pb[:, 0:wn],
                            AF.Identity, bias=cbf[:, 0:1])
                    bi += 1

                nc.sync.dma_start(
                    out=out_d.ap().rearrange("b c h w -> (b c) h w")
                    [:, row0:row0 + tnr, :],
                    in_=outb.rearrange("p (r w) -> p r w", w=WP)[:, :, 0:W])

    nc.compile()
    return nc


def _prep_shared(kW1, kW2, convW, convB, caW1, caW2):
    cwbd = np.zeros((P, P), np.float32)
    cwbd[0:C, 0:C] = convW.T
    cwbd[C:P, C:P] = convW.T
    return {
        "caW2T": np.ascontiguousarray(caW2.T),
        "convWbd16": cwbd.astype(ml_dtypes.bfloat16),
        "convB2f": np.tile(convB, 2)[:, None].astype(np.float32),
    }


def kernel(x, d, kW1, kW2, convW, convB, caW1, caW2, _trace=False):
    x = np.asarray(x, np.float32).astype(ml_dtypes.bfloat16)
    d = np.asarray(d, np.float32)
    if "nc" not in _CACHE:
        _CACHE["nc"] = _build()
    nc = _CACHE["nc"]

    shared = _prep_shared(np.asarray(kW1, np.float32),
                          np.asarray(kW2, np.float32),
                          np.asarray(convW, np.float32),
                          np.asarray(convB, np.float32),
                          np.asarray(caW1, np.float32),
                          np.asarray(caW2, np.float32))
    kW1 = np.asarray(kW1, np.float32)
    kW2 = np.asarray(kW2, np.float32)
    caW1 = np.asarray(caW1, np.float32)
    in_maps = []
    for c in range(NCORES):
        sl = slice(c * BL, (c + 1) * BL)
        m = dict(shared)
        m["x"] = np.ascontiguousarray(x[sl])
        m["wpack"] = np.ascontiguousarray(
            np.concatenate([kW1.T, kW2.T, caW1.T, d[sl].T], axis=1))
        in_maps.append(m)

    res = run_bass_kernel_spmd(nc, in_maps, core_ids=list(range(NCORES)),
                               trace=_trace)
    out = np.concatenate([r["out"] for r in res.results], axis=0)
    if _trace:
        return out, res
    return out
